# revision 17
# baseline (speedup 1.0000x reference)
"""Trainium2 Bass kernel for ViT-style attention with decomposed relative position bias.

Problem: x(1,64,64,768) -> qkv proj -> 12-head attention with rel_pos_h/rel_pos_w
decomposed bias -> softmax -> out proj.  N=4096 tokens, hd=64.

Sharding: 8 cores = 4 head-groups (3 heads each) x 2 query-blocks (2048 queries).
Each core computes K^T/V for its 3 heads over all 4096 tokens (replicated within
the head-group pair), Q for its query block, attention, and a partial output
projection (its heads' channel slice).  Host sums the 4 head-group partials per
query block and adds proj_b.

Device layout choices:
- Scores computed transposed: [keys(partition), queries(free)] so both the QK^T
  and attn@V matmuls need no transposes anywhere.
- rel_h is folded into the scores matmul for free via contraction augmentation
  (K=64 -> 128): stationary = [scale*k ; e_kh], moving = [q ; RH^T].
- rel_w enters via exp-split: E = exp(qk+rel_h) * exp(rel_w); the second factor
  is a per-head [128, 2048] bf16 tile broadcast over key-chunks.
- Softmax denominators come free from a 65th ones-column on the V stationary.
- Normalization: 1/d via ACT Ln+Exp on the [1, q] row, broadcast to 64
  partitions with a K=1 fp32 matmul, then one DVE multiply.
- Matmuls in fp32r (full PE speed at N>=512, ~1.5e-4 rel err); E and V in bf16.
"""

import numpy as np
import ml_dtypes

NH, HD, C, H, W = 12, 64, 768, 64, 64
N = H * W            # 4096
G, QB = 4, 2         # head groups x query blocks = 8 cores
HPG = NH // G        # 3 heads per group
QL = N // QB         # 2048 queries per block
SCALE = HD ** -0.5

_prog_cache = {}


def _round_f32r(x):
    hi = x.astype(ml_dtypes.bfloat16).astype(np.float32)
    lo = (x - hi).astype(ml_dtypes.bfloat16).astype(np.float32)
    return np.ascontiguousarray(hi + lo)


def _pack6(w):
    # (768, M) -> [128, 6*M]: chunk c of the contraction at cols [c*M:(c+1)*M]
    m = w.shape[1]
    return np.ascontiguousarray(w.reshape(6, 128, m).transpose(1, 0, 2).reshape(128, 6 * m))


def _build_program(taps=False):
    import concourse.bacc as bacc
    import concourse.mybir as mybir
    import concourse.tile as tile
    from contextlib import ExitStack

    f32 = mybir.dt.float32
    f32r = mybir.dt.float32r
    bf16 = mybir.dt.bfloat16
    AF = mybir.ActivationFunctionType
    ADD = mybir.AluOpType.add

    nc = bacc.Bacc("TRN2", target_bir_lowering=False, debug=False)

    XT = nc.dram_tensor("xt", [C, N], f32r, kind="ExternalInput")
    XTQ = nc.dram_tensor("xtq", [C, QL], f32r, kind="ExternalInput")
    WA = nc.dram_tensor("wa", [128, 768], f32r, kind="ExternalInput")
    WC = nc.dram_tensor("wc", [128, 384], f32r, kind="ExternalInput")
    WVB = nc.dram_tensor("wvb", [128, 6 * 192], bf16, kind="ExternalInput")
    BVB = nc.dram_tensor("bvb", [128, 192], bf16, kind="ExternalInput")
    WQA = nc.dram_tensor("wqa", [128, 768], f32r, kind="ExternalInput")
    WQB = nc.dram_tensor("wqb", [128, 384], f32r, kind="ExternalInput")
    PW1 = nc.dram_tensor("pw1", [128, 768], f32r, kind="ExternalInput")
    PW2 = nc.dram_tensor("pw2", [64, 768], f32r, kind="ExternalInput")
    BA = nc.dram_tensor("ba", [128, 1], f32, kind="ExternalInput")
    BC_ = nc.dram_tensor("bc", [64, 1], f32, kind="ExternalInput")
    BQA = nc.dram_tensor("bqa", [128, 1], f32, kind="ExternalInput")
    BQB = nc.dram_tensor("bqb", [64, 1], f32, kind="ExternalInput")
    RHT = nc.dram_tensor("rht", [64, 32 * 64], bf16, kind="ExternalInput")
    RWT = nc.dram_tensor("rwt", [64, 64 * 64], bf16, kind="ExternalInput")
    IDKH = nc.dram_tensor("idkh", [64, N], f32r, kind="ExternalInput")
    OUT = nc.dram_tensor("out", [C, QL], f32, kind="ExternalOutput")

    VSTRIDE = HPG * 80  # 240 cols per token-tile in VN (80 per head: 64 V + 1 ones + 15 pad)
    if taps:
        TKAUG = nc.dram_tensor("t_kaug0", [128, N], f32r, kind="ExternalOutput")
        TQAUG = nc.dram_tensor("t_qaug0", [128, QL], f32r, kind="ExternalOutput")
        TEW2 = nc.dram_tensor("t_ew20", [128, QL], bf16, kind="ExternalOutput")
        TVN = nc.dram_tensor("t_vn", [128, 32 * VSTRIDE], bf16, kind="ExternalOutput")
        TE2 = nc.dram_tensor("t_e2", [128, 1024], bf16, kind="ExternalOutput")
        TOP = nc.dram_tensor("t_op", [65, 1024], f32, kind="ExternalOutput")
        TPRJ = nc.dram_tensor("t_prja", [128, QL], f32r, kind="ExternalOutput")

    with tile.TileContext(nc) as tc, ExitStack() as es:
        const = es.enter_context(tc.tile_pool(name="const", bufs=1))
        big = es.enter_context(tc.tile_pool(name="big", bufs=1))
        xp = es.enter_context(tc.tile_pool(name="xp", bufs=2))
        xbp = es.enter_context(tc.tile_pool(name="xbp", bufs=1))
        p1 = es.enter_context(tc.tile_pool(name="p1", bufs=2, space="PSUM"))
        scp = es.enter_context(tc.tile_pool(name="sc", bufs=2, space="PSUM"))
        avp = es.enter_context(tc.tile_pool(name="av", bufs=1, space="PSUM"))
        ep = es.enter_context(tc.tile_pool(name="ep", bufs=2))
        nrm = es.enter_context(tc.tile_pool(name="nrm", bufs=1))
        ewfp = es.enter_context(tc.tile_pool(name="ewf", bufs=1))

        # ---- persistent tiles ----
        wA_t = const.tile([128, 768], f32r, tag="wA", name="wA")
        wC_t = const.tile([128, 384], f32r, tag="wC", name="wC")
        wvb_t = const.tile([128, 6 * 192], bf16, tag="wvb", name="wvb")
        bvb_t = const.tile([128, 192], bf16, tag="bvb", name="bvb")
        wqA_t = const.tile([128, 768], f32r, tag="wqA", name="wqA")
        wqB_t = const.tile([128, 384], f32r, tag="wqB", name="wqB")
        pw1_t = const.tile([128, 768], f32r, tag="pw1", name="pw1")
        pw2_t = const.tile([64, 768], f32r, tag="pw2", name="pw2")
        bA_t = const.tile([128, 1], f32, tag="bA", name="bA")
        bC_t = const.tile([64, 1], f32, tag="bC", name="bC")
        bqA_t = const.tile([128, 1], f32, tag="bqA", name="bqA")
        bqB_t = const.tile([64, 1], f32, tag="bqB", name="bqB")
        rhT_t = const.tile([64, 32 * 64], bf16, tag="rhT", name="rhT")
        rwT_t = const.tile([64, 64 * 64], bf16, tag="rwT", name="rwT")
        ones1 = const.tile([1, 64], f32, tag="ones1", name="ones1")

        for t_, d_ in [(wA_t, WA), (wC_t, WC), (wvb_t, WVB), (bvb_t, BVB),
                       (wqA_t, WQA), (wqB_t, WQB), (pw1_t, PW1), (pw2_t, PW2),
                       (bA_t, BA), (bC_t, BC_), (bqA_t, BQA), (bqB_t, BQB),
                       (rhT_t, RHT), (rwT_t, RWT)]:
            nc.sync.dma_start(t_[:], d_.ap())
        nc.vector.memset(ones1[:], 1.0)

        KAUG = [big.tile([128, N], f32r, tag=f"kaug{h}", name=f"kaug{h}") for h in range(HPG)]
        QAUG = [big.tile([128, QL], f32r, tag=f"qaug{h}", name=f"qaug{h}") for h in range(HPG)]
        
        EW2 = [big.tile([128, QL], bf16, tag=f"ew2{h}", name=f"ew2{h}") for h in range(HPG)]
        VN = big.tile([128, 32 * VSTRIDE], bf16, tag="vn", name="vn")
        PRJA = big.tile([128, QL], f32r, tag="prja", name="prja")
        PRJB = big.tile([64, QL], f32r, tag="prjb", name="prjb")

        for h in range(HPG):
            nc.sync.dma_start(KAUG[h][64:128, :], IDKH.ap())
        vn3 = VN[:].rearrange("p (t c) -> p t c", c=VSTRIDE)
        for h in range(HPG):
            nc.vector.memset(vn3[:, :, 64 + 80 * h], 1.0)

        # ---- Q projection over this core's block ----
        for t in range(4):
            xc = []
            for c in range(6):
                xt_ = xp.tile([128, 512], f32r, tag=f"x{c}", name=f"x{c}")
                nc.sync.dma_start(xt_[:], XTQ.ap()[128 * c:128 * c + 128, 512 * t:512 * t + 512])
                xc.append(xt_)
            sl = slice(512 * t, 512 * t + 512)
            ps = p1.tile([128, 512], f32, tag="p1", name="p1")
            for c in range(6):
                nc.tensor.matmul(ps[:], wqA_t[:, 128 * c:128 * c + 128], xc[c][:],
                                 start=(c == 0), stop=(c == 5))
            nc.vector.tensor_scalar(QAUG[0][0:64, sl], ps[0:64, :], bqA_t[0:64, :], None, ADD)
            nc.vector.tensor_scalar(QAUG[1][0:64, sl], ps[64:128, :], bqA_t[64:128, :], None, ADD)
            ps2 = p1.tile([64, 512], f32, tag="p1", name="p1b")
            for c in range(6):
                nc.tensor.matmul(ps2[:], wqB_t[:, 64 * c:64 * c + 64], xc[c][:],
                                 start=(c == 0), stop=(c == 5))
            nc.vector.tensor_scalar(QAUG[2][0:64, sl], ps2[:], bqB_t[:], None, ADD)
        # ---- RH^T into QAUG rows 64-127; RW^T -> exp -> EW2 (bf16 matmuls) ----
        for h in range(HPG):
            qb16 = ewfp.tile([64, QL], bf16, tag="qb16", name="qb16")
            nc.vector.tensor_copy(qb16[:], QAUG[h][0:64, :])
            for i4 in range(8):
                ps = p1.tile([64, 256], f32, tag="p1", name="p1rh")
                for k in range(4):
                    i = 4 * i4 + k
                    nc.tensor.matmul(ps[:, 64 * k:64 * k + 64],
                                     rhT_t[:, 64 * i:64 * i + 64],
                                     qb16[:, 64 * i:64 * i + 64],
                                     start=True, stop=True)
                nc.vector.tensor_copy(QAUG[h][64:128, 256 * i4:256 * i4 + 256], ps[:])
            ewf = ewfp.tile([64, QL], bf16, tag="ewf", name="ewf")
            qa = qb16[:].rearrange("p (i w) -> p w i", w=64)
            ef = ewf[:].rearrange("p (i w) -> p w i", w=64)
            for w4 in range(16):
                ps = p1.tile([64, 128], f32, tag="p1", name="p1rw")
                for k in range(4):
                    w = 4 * w4 + k
                    nc.tensor.matmul(ps[:, 32 * k:32 * k + 32],
                                     rwT_t[:, 64 * w:64 * w + 64], qa[:, w, :],
                                     start=True, stop=True)
                nc.vector.tensor_copy(
                    ef[:, 4 * w4:4 * w4 + 4, :],
                    ps[:].rearrange("p (k i) -> p k i", i=32))
            nc.scalar.activation(EW2[h][0:64, :], ewf[:], AF.Exp)
            nc.vector.tensor_copy(EW2[h][64:128, :], EW2[h][0:64, :])

        # ---- K projection + V natural layout, streaming all tokens ----
        for t in range(8):
            xc = []
            xb = []
            for c in range(6):
                xt_ = xp.tile([128, 512], f32r, tag=f"x{c}", name=f"x{c}")
                nc.sync.dma_start(xt_[:], XT.ap()[128 * c:128 * c + 128, 512 * t:512 * t + 512])
                xc.append(xt_)
                xb_ = xbp.tile([128, 512], bf16, tag=f"xb{c}", name=f"xb{c}")
                nc.gpsimd.tensor_copy(xb_[:], xt_[:])
                xb.append(xb_)
            sl = slice(512 * t, 512 * t + 512)
            ps = p1.tile([128, 512], f32, tag="p1", name="p1k")
            for c in range(6):
                nc.tensor.matmul(ps[:], wA_t[:, 128 * c:128 * c + 128], xc[c][:],
                                 start=(c == 0), stop=(c == 5))
            nc.vector.tensor_scalar(KAUG[0][0:64, sl], ps[0:64, :], bA_t[0:64, :], None, ADD)
            nc.vector.tensor_scalar(KAUG[1][0:64, sl], ps[64:128, :], bA_t[64:128, :], None, ADD)
            ps2 = p1.tile([64, 512], f32, tag="p1", name="p1k2")
            for c in range(6):
                nc.tensor.matmul(ps2[:], wC_t[:, 64 * c:64 * c + 64], xc[c][:],
                                 start=(c == 0), stop=(c == 5))
            nc.vector.tensor_scalar(KAUG[2][0:64, sl], ps2[:], bC_t[:], None, ADD)
            for s in range(4):
                tt = 4 * t + s
                pv = p1.tile([128, 192], f32, tag="p1", name="p1v")
                for c in range(6):
                    nc.tensor.matmul(pv[:], xb[c][:, 128 * s:128 * s + 128],
                                     wvb_t[:, 192 * c:192 * c + 192],
                                     start=(c == 0), stop=(c == 5))
                vdst = VN[:, VSTRIDE * tt:VSTRIDE * tt + VSTRIDE].rearrange(
                    "p (h c) -> p h c", c=80)[:, :, 0:64]
                nc.vector.tensor_tensor(
                    vdst, pv[:].rearrange("p (h c) -> p h c", c=64),
                    bvb_t[:].rearrange("p (h c) -> p h c", c=64), ADD)

        # ---- attention (chases the K/V stream via Tile deps) ----
        for h in range(HPG):
            for qc in range(2):
                q0 = 1024 * qc
                O_ps = avp.tile([65, 1024], f32, tag="av", name="av")
                for kc in range(32):
                    S_ps = scp.tile([128, 1024], f32, tag="sc", name="sc")
                    for s in range(2):
                        nc.tensor.matmul(S_ps[:, 512 * s:512 * s + 512],
                                         KAUG[h][:, 128 * kc:128 * kc + 128],
                                         QAUG[h][:, q0 + 512 * s:q0 + 512 * s + 512],
                                         start=True, stop=True)
                    E1 = ep.tile([128, 1024], bf16, tag="e1", name="e1")
                    nc.scalar.activation(E1[:], S_ps[:], AF.Exp)
                    E2 = ep.tile([128, 1024], bf16, tag="e2", name="e2")
                    nc.vector.tensor_mul(E2[:], E1[:], EW2[h][:, q0:q0 + 1024])
                    if taps and h == 0 and qc == 0 and kc == 0:
                        nc.sync.dma_start(TE2.ap(), E2[:])
                    for s in range(2):
                        nc.tensor.matmul(O_ps[:, 512 * s:512 * s + 512],
                                         VN[:, VSTRIDE * kc + 80 * h:VSTRIDE * kc + 80 * h + 65],
                                         E2[:, 512 * s:512 * s + 512],
                                         start=(kc == 0), stop=(kc == 31))
                if taps and h == 0 and qc == 0:
                    topst = nrm.tile([65, 1024], f32, tag="topst", name="topst")
                    nc.vector.tensor_copy(topst[:], O_ps[:])
                    nc.sync.dma_start(TOP.ap(), topst[:])
                # copy O_ps to SBUF immediately so the next chunk can reuse the
                # PSUM slot; normalize from the SBUF copy
                O_sb = const.tile([65, 1024], f32, tag="rwT", name="osb")  # reuse rwT slot (dead after phase 1)
                nc.vector.tensor_copy(O_sb[:], O_ps[:])
                # normalize: 1/d via Ln+Exp (f32r out), broadcast via K=1 f32r matmul
                ln_t = nrm.tile([1, 1024], f32, tag="ln", name="ln")
                nc.scalar.activation(ln_t[:], O_sb[64:65, :], AF.Ln)
                rec_t = nrm.tile([1, 1024], f32r, tag="rec", name="rec")
                nc.scalar.activation(rec_t[:], ln_t[:], AF.Exp, scale=-1.0)
                B_ps = scp.tile([64, 1024], f32, tag="sc", name="bcb")
                onesr = nrm.tile([1, 64], f32r, tag="onesr", name="onesr")
                nc.vector.tensor_copy(onesr[:], ones1[:])
                for s in range(2):
                    nc.tensor.matmul(B_ps[:, 512 * s:512 * s + 512], onesr[:],
                                     rec_t[:, 512 * s:512 * s + 512], start=True, stop=True)
                B_sb = nrm.tile([64, 1024], f32, tag="bcs", name="bcs")
                nc.vector.tensor_copy(B_sb[:], B_ps[:])
                dst = PRJA[64 * h:64 * h + 64, q0:q0 + 1024] if h < 2 else PRJB[0:64, q0:q0 + 1024]
                nc.vector.tensor_mul(dst, O_sb[0:64, :], B_sb[:])

        if taps:
            nc.sync.dma_start(TKAUG.ap(), KAUG[0][:])
            nc.sync.dma_start(TQAUG.ap(), QAUG[0][:])
            nc.sync.dma_start(TEW2.ap(), EW2[0][:])
            nc.sync.dma_start(TVN.ap(), VN[:])
            nc.sync.dma_start(TPRJ.ap(), PRJA[:])

        # ---- output projection (partial over this group's channels) ----
        with tc.tile_pool(name="ost", bufs=1) as ostp:
            for m in range(6):
                for qc in range(2):
                    q0 = 1024 * qc
                    ps = scp.tile([128, 1024], f32, tag="sc", name="po")
                    for s in range(2):
                        nc.tensor.matmul(ps[:, 512 * s:512 * s + 512],
                                         pw1_t[:, 128 * m:128 * m + 128],
                                         PRJA[:, q0 + 512 * s:q0 + 512 * s + 512],
                                         start=True, stop=False)
                        nc.tensor.matmul(ps[:, 512 * s:512 * s + 512],
                                         pw2_t[:, 128 * m:128 * m + 128],
                                         PRJB[:, q0 + 512 * s:q0 + 512 * s + 512],
                                         start=False, stop=True)
                    ost = ostp.tile([128, 1024], f32, tag="ost", name="ost")
                    nc.vector.tensor_copy(ost[:], ps[:])
                    nc.sync.dma_start(OUT.ap()[128 * m:128 * m + 128, q0:q0 + 1024], ost[:])

    nc.compile()
    return nc


def _host_inputs(x, qkv_w, qkv_b, proj_w, rel_pos_h, rel_pos_w):
    """Build the 8 per-core input maps."""
    xmat = np.ascontiguousarray(x.reshape(N, C))
    xT = _round_f32r(xmat.T.astype(np.float32))

    idx = np.arange(64)[:, None] - np.arange(64)[None, :] + 63
    rh_g = rel_pos_h[idx]            # (h, kh, c)
    rw_g = rel_pos_w[idx]            # (w, kw, c)
    rwT = np.ascontiguousarray(rw_g.transpose(2, 0, 1).reshape(64, 64 * 64)).astype(ml_dtypes.bfloat16)
    idkh = _round_f32r(
        (np.arange(64)[:, None] == (np.arange(N)[None, :] // 64)).astype(np.float32))

    in_maps = []
    for core in range(8):
        g, j = core // QB, core % QB
        cs = slice(192 * g, 192 * g + 192)
        wq = qkv_w[:, 0 * C:1 * C][:, cs]
        wk = qkv_w[:, 1 * C:2 * C][:, cs] * SCALE
        wv = qkv_w[:, 2 * C:3 * C][:, cs]
        bq = qkv_b[0 * C:1 * C][cs]
        bk = qkv_b[1 * C:2 * C][cs] * SCALE
        bv = qkv_b[2 * C:3 * C][cs]

        h0 = 32 * j
        rhT = np.ascontiguousarray(rh_g[h0:h0 + 32].transpose(2, 0, 1).reshape(64, 32 * 64)).astype(ml_dtypes.bfloat16)

        m = {
            "xt": xT,
            "xtq": np.ascontiguousarray(xT[:, QL * j:QL * j + QL]),
            "wa": _round_f32r(_pack6(wk[:, 0:128])),
            "wc": _round_f32r(_pack6(wk[:, 128:192])),
            "wvb": _pack6(wv).astype(ml_dtypes.bfloat16),
            "bvb": np.ascontiguousarray(
                np.broadcast_to(bv[None, :], (128, 192))).astype(ml_dtypes.bfloat16),
            "wqa": _round_f32r(_pack6(wq[:, 0:128])),
            "wqb": _round_f32r(_pack6(wq[:, 128:192])),
            "pw1": _round_f32r(proj_w[cs][0:128, :]),
            "pw2": _round_f32r(proj_w[cs][128:192, :]),
            "ba": np.ascontiguousarray(bk[0:128, None].astype(np.float32)),
            "bc": np.ascontiguousarray(bk[128:192, None].astype(np.float32)),
            "bqa": np.ascontiguousarray(bq[0:128, None].astype(np.float32)),
            "bqb": np.ascontiguousarray(bq[128:192, None].astype(np.float32)),
            "rht": rhT,
            "rwt": rwT,
            "idkh": idkh,
        }
        in_maps.append(m)
    return in_maps


def kernel(x, qkv_w, qkv_b, proj_w, proj_b, rel_pos_h, rel_pos_w):
    from concourse.bass_utils import run_bass_kernel_spmd

    x = np.asarray(x, dtype=np.float32)
    qkv_w = np.asarray(qkv_w, dtype=np.float32)
    qkv_b = np.asarray(qkv_b, dtype=np.float32)
    proj_w = np.asarray(proj_w, dtype=np.float32)
    proj_b = np.asarray(proj_b, dtype=np.float32)
    rel_pos_h = np.asarray(rel_pos_h, dtype=np.float32)
    rel_pos_w = np.asarray(rel_pos_w, dtype=np.float32)

    if "nc" not in _prog_cache:
        _prog_cache["nc"] = _build_program()
    nc = _prog_cache["nc"]

    in_maps = _host_inputs(x, qkv_w, qkv_b, proj_w, rel_pos_h, rel_pos_w)
    res = run_bass_kernel_spmd(nc, in_maps, core_ids=list(range(8)))

    out = np.zeros((N, C), dtype=np.float32)
    for core in range(8):
        g, j = core // QB, core % QB
        out[QL * j:QL * j + QL, :] += res.results[core]["out"].T
    out += proj_b[None, :]
    return out.reshape(1, H, W, C).astype(np.float32)


# revision 18
# speedup vs baseline: 1.0450x; 1.0450x over previous
"""Trainium2 Bass kernel for ViT-style attention with decomposed relative position bias.

Problem: x(1,64,64,768) -> qkv proj -> 12-head attention with rel_pos_h/rel_pos_w
decomposed bias -> softmax -> out proj.  N=4096 tokens, hd=64.

Sharding: 8 cores = 4 head-groups (3 heads each) x 2 query-blocks (2048 queries).
Each core computes K^T/V for its 3 heads over all 4096 tokens (replicated within
the head-group pair), Q for its query block, attention, and a partial output
projection (its heads' channel slice).  Host sums the 4 head-group partials per
query block and adds proj_b.

Device layout choices:
- Scores computed transposed: [keys(partition), queries(free)] so both the QK^T
  and attn@V matmuls need no transposes anywhere.
- rel_h is folded into the scores matmul for free via contraction augmentation
  (K=64 -> 128): stationary = [scale*k ; e_kh], moving = [q ; RH^T].
- rel_w enters via exp-split: E = exp(qk+rel_h) * exp(rel_w); the second factor
  is a per-head [128, 2048] bf16 tile broadcast over key-chunks.
- Softmax denominators come free from a 65th ones-column on the V stationary.
- Normalization: 1/d via ACT Ln+Exp on the [1, q] row, broadcast to 64
  partitions with a K=1 fp32 matmul, then one DVE multiply.
- Matmuls in fp32r (full PE speed at N>=512, ~1.5e-4 rel err); E and V in bf16.
"""

import numpy as np
import ml_dtypes

NH, HD, C, H, W = 12, 64, 768, 64, 64
N = H * W            # 4096
G, QB = 4, 2         # head groups x query blocks = 8 cores
HPG = NH // G        # 3 heads per group
QL = N // QB         # 2048 queries per block
SCALE = HD ** -0.5

_prog_cache = {}


def _round_f32r(x):
    hi = x.astype(ml_dtypes.bfloat16).astype(np.float32)
    lo = (x - hi).astype(ml_dtypes.bfloat16).astype(np.float32)
    return np.ascontiguousarray(hi + lo)


def _pack6(w):
    # (768, M) -> [128, 6*M]: chunk c of the contraction at cols [c*M:(c+1)*M]
    m = w.shape[1]
    return np.ascontiguousarray(w.reshape(6, 128, m).transpose(1, 0, 2).reshape(128, 6 * m))


def _build_program(taps=False):
    import concourse.bacc as bacc
    import concourse.mybir as mybir
    import concourse.tile as tile
    from contextlib import ExitStack

    f32 = mybir.dt.float32
    f32r = mybir.dt.float32r
    bf16 = mybir.dt.bfloat16
    AF = mybir.ActivationFunctionType
    ADD = mybir.AluOpType.add

    nc = bacc.Bacc("TRN2", target_bir_lowering=False, debug=False)

    XT = nc.dram_tensor("xt", [C, N], f32r, kind="ExternalInput")
    XTQ = nc.dram_tensor("xtq", [C, QL], f32r, kind="ExternalInput")
    WA = nc.dram_tensor("wa", [128, 768], f32r, kind="ExternalInput")
    WC = nc.dram_tensor("wc", [128, 384], f32r, kind="ExternalInput")
    WVB = nc.dram_tensor("wvb", [128, 6 * 192], bf16, kind="ExternalInput")
    BVB = nc.dram_tensor("bvb", [128, 192], bf16, kind="ExternalInput")
    WQA = nc.dram_tensor("wqa", [128, 768], f32r, kind="ExternalInput")
    WQB = nc.dram_tensor("wqb", [128, 384], f32r, kind="ExternalInput")
    PW1 = nc.dram_tensor("pw1", [128, 768], f32r, kind="ExternalInput")
    PW2 = nc.dram_tensor("pw2", [64, 768], f32r, kind="ExternalInput")
    BA = nc.dram_tensor("ba", [128, 1], f32, kind="ExternalInput")
    BC_ = nc.dram_tensor("bc", [64, 1], f32, kind="ExternalInput")
    BQA = nc.dram_tensor("bqa", [128, 1], f32, kind="ExternalInput")
    BQB = nc.dram_tensor("bqb", [64, 1], f32, kind="ExternalInput")
    RHT = nc.dram_tensor("rht", [64, 32 * 64], bf16, kind="ExternalInput")
    RWT = nc.dram_tensor("rwt", [64, 64 * 64], bf16, kind="ExternalInput")
    IDKH = nc.dram_tensor("idkh", [64, N], f32r, kind="ExternalInput")
    OUT = nc.dram_tensor("out", [C, QL], f32, kind="ExternalOutput")

    VSTRIDE = HPG * 80  # 240 cols per token-tile in VN (80 per head: 64 V + 1 ones + 15 pad)
    if taps:
        TKAUG = nc.dram_tensor("t_kaug0", [128, N], f32r, kind="ExternalOutput")
        TQAUG = nc.dram_tensor("t_qaug0", [128, QL], f32r, kind="ExternalOutput")
        TEW2 = nc.dram_tensor("t_ew20", [128, QL], bf16, kind="ExternalOutput")
        TVN = nc.dram_tensor("t_vn", [128, 32 * VSTRIDE], bf16, kind="ExternalOutput")
        TE2 = nc.dram_tensor("t_e2", [128, 1024], bf16, kind="ExternalOutput")
        TOP = nc.dram_tensor("t_op", [65, 1024], f32, kind="ExternalOutput")
        TPRJ = nc.dram_tensor("t_prja", [128, QL], f32r, kind="ExternalOutput")

    with tile.TileContext(nc) as tc, ExitStack() as es:
        const = es.enter_context(tc.tile_pool(name="const", bufs=1))
        big = es.enter_context(tc.tile_pool(name="big", bufs=1))
        xp = es.enter_context(tc.tile_pool(name="xp", bufs=2))
        xbp = es.enter_context(tc.tile_pool(name="xbp", bufs=1))
        p1 = es.enter_context(tc.tile_pool(name="p1", bufs=2, space="PSUM"))
        scp = es.enter_context(tc.tile_pool(name="sc", bufs=3, space="PSUM"))
        ep = es.enter_context(tc.tile_pool(name="ep", bufs=2))
        nrm = es.enter_context(tc.tile_pool(name="nrm", bufs=1))
        ewfp = es.enter_context(tc.tile_pool(name="ewf", bufs=1))

        # ---- persistent tiles ----
        wA_t = const.tile([128, 768], f32r, tag="wA", name="wA")
        wC_t = const.tile([128, 384], f32r, tag="wC", name="wC")
        wvb_t = const.tile([128, 6 * 192], bf16, tag="wvb", name="wvb")
        bvb_t = const.tile([128, 192], bf16, tag="bvb", name="bvb")
        wqA_t = const.tile([128, 768], f32r, tag="wqA", name="wqA")
        wqB_t = const.tile([128, 384], f32r, tag="wqB", name="wqB")
        pw1_t = const.tile([128, 768], f32r, tag="pw1", name="pw1")
        pw2_t = const.tile([64, 768], f32r, tag="pw2", name="pw2")
        bA_t = const.tile([128, 1], f32, tag="bA", name="bA")
        bC_t = const.tile([64, 1], f32, tag="bC", name="bC")
        bqA_t = const.tile([128, 1], f32, tag="bqA", name="bqA")
        bqB_t = const.tile([64, 1], f32, tag="bqB", name="bqB")
        rhT_t = const.tile([64, 32 * 64], bf16, tag="rhT", name="rhT")
        rwT_t = const.tile([64, 64 * 64], bf16, tag="rwT", name="rwT")
        ones1 = const.tile([1, 64], f32, tag="ones1", name="ones1")

        for t_, d_ in [(wA_t, WA), (wC_t, WC), (wvb_t, WVB), (bvb_t, BVB),
                       (wqA_t, WQA), (wqB_t, WQB), (pw1_t, PW1), (pw2_t, PW2),
                       (bA_t, BA), (bC_t, BC_), (bqA_t, BQA), (bqB_t, BQB),
                       (rhT_t, RHT), (rwT_t, RWT)]:
            nc.sync.dma_start(t_[:], d_.ap())
        nc.vector.memset(ones1[:], 1.0)

        KAUG = [big.tile([128, N], f32r, tag=f"kaug{h}", name=f"kaug{h}") for h in range(HPG)]
        QAUG = [big.tile([128, QL], f32r, tag=f"qaug{h}", name=f"qaug{h}") for h in range(HPG)]
        
        EW2 = [big.tile([128, QL], bf16, tag=f"ew2{h}", name=f"ew2{h}") for h in range(HPG)]
        VN = big.tile([128, 32 * VSTRIDE], bf16, tag="vn", name="vn")
        PRJA = big.tile([128, QL], f32r, tag="prja", name="prja")
        PRJB = big.tile([64, QL], f32r, tag="prjb", name="prjb")

        for h in range(HPG):
            nc.sync.dma_start(KAUG[h][64:128, :], IDKH.ap())
        vn3 = VN[:].rearrange("p (t c) -> p t c", c=VSTRIDE)
        for h in range(HPG):
            nc.vector.memset(vn3[:, :, 64 + 80 * h], 1.0)

        # ---- Q projection over this core's block ----
        for t in range(4):
            xc = []
            for c in range(6):
                xt_ = xp.tile([128, 512], f32r, tag=f"x{c}", name=f"x{c}")
                nc.sync.dma_start(xt_[:], XTQ.ap()[128 * c:128 * c + 128, 512 * t:512 * t + 512])
                xc.append(xt_)
            sl = slice(512 * t, 512 * t + 512)
            ps = p1.tile([128, 512], f32, tag="p1", name="p1")
            for c in range(6):
                nc.tensor.matmul(ps[:], wqA_t[:, 128 * c:128 * c + 128], xc[c][:],
                                 start=(c == 0), stop=(c == 5))
            nc.vector.tensor_scalar(QAUG[0][0:64, sl], ps[0:64, :], bqA_t[0:64, :], None, ADD)
            nc.vector.tensor_scalar(QAUG[1][0:64, sl], ps[64:128, :], bqA_t[64:128, :], None, ADD)
            ps2 = p1.tile([64, 512], f32, tag="p1", name="p1b")
            for c in range(6):
                nc.tensor.matmul(ps2[:], wqB_t[:, 64 * c:64 * c + 64], xc[c][:],
                                 start=(c == 0), stop=(c == 5))
            nc.vector.tensor_scalar(QAUG[2][0:64, sl], ps2[:], bqB_t[:], None, ADD)
        # ---- RH^T into QAUG rows 64-127; RW^T -> exp -> EW2 (bf16 matmuls) ----
        for h in range(HPG):
            qb16 = ewfp.tile([64, QL], bf16, tag="qb16", name="qb16")
            nc.vector.tensor_copy(qb16[:], QAUG[h][0:64, :])
            for i4 in range(8):
                ps = p1.tile([64, 256], f32, tag="p1", name="p1rh")
                for k in range(4):
                    i = 4 * i4 + k
                    nc.tensor.matmul(ps[:, 64 * k:64 * k + 64],
                                     rhT_t[:, 64 * i:64 * i + 64],
                                     qb16[:, 64 * i:64 * i + 64],
                                     start=True, stop=True)
                nc.vector.tensor_copy(QAUG[h][64:128, 256 * i4:256 * i4 + 256], ps[:])
            ewf = ewfp.tile([64, QL], bf16, tag="ewf", name="ewf")
            qa = qb16[:].rearrange("p (i w) -> p w i", w=64)
            ef = ewf[:].rearrange("p (i w) -> p w i", w=64)
            for w4 in range(16):
                ps = p1.tile([64, 128], f32, tag="p1", name="p1rw")
                for k in range(4):
                    w = 4 * w4 + k
                    nc.tensor.matmul(ps[:, 32 * k:32 * k + 32],
                                     rwT_t[:, 64 * w:64 * w + 64], qa[:, w, :],
                                     start=True, stop=True)
                nc.vector.tensor_copy(
                    ef[:, 4 * w4:4 * w4 + 4, :],
                    ps[:].rearrange("p (k i) -> p k i", i=32))
            nc.scalar.activation(EW2[h][0:64, :], ewf[:], AF.Exp)
            nc.vector.tensor_copy(EW2[h][64:128, :], EW2[h][0:64, :])

        # ---- K projection + V natural layout, streaming all tokens ----
        for t in range(8):
            xc = []
            xb = []
            for c in range(6):
                xt_ = xp.tile([128, 512], f32r, tag=f"x{c}", name=f"x{c}")
                nc.sync.dma_start(xt_[:], XT.ap()[128 * c:128 * c + 128, 512 * t:512 * t + 512])
                xc.append(xt_)
                xb_ = xbp.tile([128, 512], bf16, tag=f"xb{c}", name=f"xb{c}")
                nc.gpsimd.tensor_copy(xb_[:], xt_[:])
                xb.append(xb_)
            sl = slice(512 * t, 512 * t + 512)
            ps = p1.tile([128, 512], f32, tag="p1", name="p1k")
            for c in range(6):
                nc.tensor.matmul(ps[:], wA_t[:, 128 * c:128 * c + 128], xc[c][:],
                                 start=(c == 0), stop=(c == 5))
            nc.vector.tensor_scalar(KAUG[0][0:64, sl], ps[0:64, :], bA_t[0:64, :], None, ADD)
            nc.vector.tensor_scalar(KAUG[1][0:64, sl], ps[64:128, :], bA_t[64:128, :], None, ADD)
            ps2 = p1.tile([64, 512], f32, tag="p1", name="p1k2")
            for c in range(6):
                nc.tensor.matmul(ps2[:], wC_t[:, 64 * c:64 * c + 64], xc[c][:],
                                 start=(c == 0), stop=(c == 5))
            nc.vector.tensor_scalar(KAUG[2][0:64, sl], ps2[:], bC_t[:], None, ADD)
            for s in range(4):
                tt = 4 * t + s
                pv = p1.tile([128, 192], f32, tag="p1", name="p1v")
                for c in range(6):
                    nc.tensor.matmul(pv[:], xb[c][:, 128 * s:128 * s + 128],
                                     wvb_t[:, 192 * c:192 * c + 192],
                                     start=(c == 0), stop=(c == 5))
                vdst = VN[:, VSTRIDE * tt:VSTRIDE * tt + VSTRIDE].rearrange(
                    "p (h c) -> p h c", c=80)[:, :, 0:64]
                nc.vector.tensor_tensor(
                    vdst, pv[:].rearrange("p (h c) -> p h c", c=64),
                    bvb_t[:].rearrange("p (h c) -> p h c", c=64), ADD)

        # ---- attention (chases the K/V stream via Tile deps) ----
        for h in range(HPG):
            for qc in range(2):
                q0 = 1024 * qc
                O_ps = scp.tile([65, 1024], f32, tag="sc", name="av")
                for kc in range(32):
                    S_ps = scp.tile([128, 1024], f32, tag="sc", name="sc")
                    for s in range(2):
                        nc.tensor.matmul(S_ps[:, 512 * s:512 * s + 512],
                                         KAUG[h][:, 128 * kc:128 * kc + 128],
                                         QAUG[h][:, q0 + 512 * s:q0 + 512 * s + 512],
                                         start=True, stop=True)
                    E1 = ep.tile([128, 1024], bf16, tag="e1", name="e1")
                    nc.scalar.activation(E1[:], S_ps[:], AF.Exp)
                    E2 = ep.tile([128, 1024], bf16, tag="e2", name="e2")
                    nc.vector.tensor_mul(E2[:], E1[:], EW2[h][:, q0:q0 + 1024])
                    if taps and h == 0 and qc == 0 and kc == 0:
                        nc.sync.dma_start(TE2.ap(), E2[:])
                    for s in range(2):
                        nc.tensor.matmul(O_ps[:, 512 * s:512 * s + 512],
                                         VN[:, VSTRIDE * kc + 80 * h:VSTRIDE * kc + 80 * h + 65],
                                         E2[:, 512 * s:512 * s + 512],
                                         start=(kc == 0), stop=(kc == 31))
                if taps and h == 0 and qc == 0:
                    topst = nrm.tile([65, 1024], f32, tag="topst", name="topst")
                    nc.vector.tensor_copy(topst[:], O_ps[:])
                    nc.sync.dma_start(TOP.ap(), topst[:])
                # copy O_ps to SBUF immediately so the next chunk can reuse the
                # PSUM slot; normalize from the SBUF copy
                O_sb = const.tile([65, 1024], f32, tag="rwT", name="osb")  # reuse rwT slot (dead after phase 1)
                nc.vector.tensor_copy(O_sb[:], O_ps[:])
                # normalize: 1/d via Ln+Exp (f32r out), broadcast via K=1 f32r matmul
                ln_t = nrm.tile([1, 1024], f32, tag="ln", name="ln")
                nc.scalar.activation(ln_t[:], O_sb[64:65, :], AF.Ln)
                rec_t = nrm.tile([1, 1024], f32r, tag="rec", name="rec")
                nc.scalar.activation(rec_t[:], ln_t[:], AF.Exp, scale=-1.0)
                B_ps = scp.tile([64, 1024], f32, tag="sc", name="bcb")
                onesr = nrm.tile([1, 64], f32r, tag="onesr", name="onesr")
                nc.vector.tensor_copy(onesr[:], ones1[:])
                for s in range(2):
                    nc.tensor.matmul(B_ps[:, 512 * s:512 * s + 512], onesr[:],
                                     rec_t[:, 512 * s:512 * s + 512], start=True, stop=True)
                B_sb = nrm.tile([64, 1024], f32, tag="bcs", name="bcs")
                nc.vector.tensor_copy(B_sb[:], B_ps[:])
                dst = PRJA[64 * h:64 * h + 64, q0:q0 + 1024] if h < 2 else PRJB[0:64, q0:q0 + 1024]
                nc.vector.tensor_mul(dst, O_sb[0:64, :], B_sb[:])

        if taps:
            nc.sync.dma_start(TKAUG.ap(), KAUG[0][:])
            nc.sync.dma_start(TQAUG.ap(), QAUG[0][:])
            nc.sync.dma_start(TEW2.ap(), EW2[0][:])
            nc.sync.dma_start(TVN.ap(), VN[:])
            nc.sync.dma_start(TPRJ.ap(), PRJA[:])

        # ---- output projection (partial over this group's channels) ----
        if True:
            for m in range(6):
                for qc in range(2):
                    q0 = 1024 * qc
                    ps = scp.tile([128, 1024], f32, tag="sc", name="po")
                    for s in range(2):
                        nc.tensor.matmul(ps[:, 512 * s:512 * s + 512],
                                         pw1_t[:, 128 * m:128 * m + 128],
                                         PRJA[:, q0 + 512 * s:q0 + 512 * s + 512],
                                         start=True, stop=False)
                        nc.tensor.matmul(ps[:, 512 * s:512 * s + 512],
                                         pw2_t[:, 128 * m:128 * m + 128],
                                         PRJB[:, q0 + 512 * s:q0 + 512 * s + 512],
                                         start=False, stop=True)
                    ost = big.tile([128, 1024], f32, tag=f"ew2{(2 * m + qc) % 3}", name="ost")
                    nc.vector.tensor_copy(ost[:], ps[:])
                    nc.sync.dma_start(OUT.ap()[128 * m:128 * m + 128, q0:q0 + 1024], ost[:])

    nc.compile()
    return nc


def _host_inputs(x, qkv_w, qkv_b, proj_w, rel_pos_h, rel_pos_w):
    """Build the 8 per-core input maps."""
    xmat = np.ascontiguousarray(x.reshape(N, C))
    xT = _round_f32r(xmat.T.astype(np.float32))

    idx = np.arange(64)[:, None] - np.arange(64)[None, :] + 63
    rh_g = rel_pos_h[idx]            # (h, kh, c)
    rw_g = rel_pos_w[idx]            # (w, kw, c)
    rwT = np.ascontiguousarray(rw_g.transpose(2, 0, 1).reshape(64, 64 * 64)).astype(ml_dtypes.bfloat16)
    idkh = _round_f32r(
        (np.arange(64)[:, None] == (np.arange(N)[None, :] // 64)).astype(np.float32))

    in_maps = []
    for core in range(8):
        g, j = core // QB, core % QB
        cs = slice(192 * g, 192 * g + 192)
        wq = qkv_w[:, 0 * C:1 * C][:, cs]
        wk = qkv_w[:, 1 * C:2 * C][:, cs] * SCALE
        wv = qkv_w[:, 2 * C:3 * C][:, cs]
        bq = qkv_b[0 * C:1 * C][cs]
        bk = qkv_b[1 * C:2 * C][cs] * SCALE
        bv = qkv_b[2 * C:3 * C][cs]

        h0 = 32 * j
        rhT = np.ascontiguousarray(rh_g[h0:h0 + 32].transpose(2, 0, 1).reshape(64, 32 * 64)).astype(ml_dtypes.bfloat16)

        m = {
            "xt": xT,
            "xtq": np.ascontiguousarray(xT[:, QL * j:QL * j + QL]),
            "wa": _round_f32r(_pack6(wk[:, 0:128])),
            "wc": _round_f32r(_pack6(wk[:, 128:192])),
            "wvb": _pack6(wv).astype(ml_dtypes.bfloat16),
            "bvb": np.ascontiguousarray(
                np.broadcast_to(bv[None, :], (128, 192))).astype(ml_dtypes.bfloat16),
            "wqa": _round_f32r(_pack6(wq[:, 0:128])),
            "wqb": _round_f32r(_pack6(wq[:, 128:192])),
            "pw1": _round_f32r(proj_w[cs][0:128, :]),
            "pw2": _round_f32r(proj_w[cs][128:192, :]),
            "ba": np.ascontiguousarray(bk[0:128, None].astype(np.float32)),
            "bc": np.ascontiguousarray(bk[128:192, None].astype(np.float32)),
            "bqa": np.ascontiguousarray(bq[0:128, None].astype(np.float32)),
            "bqb": np.ascontiguousarray(bq[128:192, None].astype(np.float32)),
            "rht": rhT,
            "rwt": rwT,
            "idkh": idkh,
        }
        in_maps.append(m)
    return in_maps


def kernel(x, qkv_w, qkv_b, proj_w, proj_b, rel_pos_h, rel_pos_w):
    from concourse.bass_utils import run_bass_kernel_spmd

    x = np.asarray(x, dtype=np.float32)
    qkv_w = np.asarray(qkv_w, dtype=np.float32)
    qkv_b = np.asarray(qkv_b, dtype=np.float32)
    proj_w = np.asarray(proj_w, dtype=np.float32)
    proj_b = np.asarray(proj_b, dtype=np.float32)
    rel_pos_h = np.asarray(rel_pos_h, dtype=np.float32)
    rel_pos_w = np.asarray(rel_pos_w, dtype=np.float32)

    if "nc" not in _prog_cache:
        _prog_cache["nc"] = _build_program()
    nc = _prog_cache["nc"]

    in_maps = _host_inputs(x, qkv_w, qkv_b, proj_w, rel_pos_h, rel_pos_w)
    res = run_bass_kernel_spmd(nc, in_maps, core_ids=list(range(8)))

    out = np.zeros((N, C), dtype=np.float32)
    for core in range(8):
        g, j = core // QB, core % QB
        out[QL * j:QL * j + QL, :] += res.results[core]["out"].T
    out += proj_b[None, :]
    return out.reshape(1, H, W, C).astype(np.float32)


# revision 20
# speedup vs baseline: 1.0857x; 1.0389x over previous
"""Trainium2 Bass kernel for ViT-style attention with decomposed relative position bias.

Problem: x(1,64,64,768) -> qkv proj -> 12-head attention with rel_pos_h/rel_pos_w
decomposed bias -> softmax -> out proj.  N=4096 tokens, hd=64.

Sharding: 8 cores = 4 head-groups (3 heads each) x 2 query-blocks (2048 queries).
Each core computes K^T/V for its 3 heads over all 4096 tokens (replicated within
the head-group pair), Q for its query block, attention, and a partial output
projection (its heads' channel slice).  Host sums the 4 head-group partials per
query block and adds proj_b.

Device layout choices:
- Scores computed transposed: [keys(partition), queries(free)] so both the QK^T
  and attn@V matmuls need no transposes anywhere.
- rel_h is folded into the scores matmul for free via contraction augmentation
  (K=64 -> 128): stationary = [scale*k ; e_kh], moving = [q ; RH^T].
- rel_w enters via exp-split: E = exp(qk+rel_h) * exp(rel_w); the second factor
  is a per-head [128, 2048] bf16 tile broadcast over key-chunks.
- Softmax denominators come free from a 65th ones-column on the V stationary.
- Normalization: 1/d via ACT Ln+Exp on the [1, q] row, broadcast to 64
  partitions with a K=1 fp32 matmul, then one DVE multiply.
- Matmuls in fp32r (full PE speed at N>=512, ~1.5e-4 rel err); E and V in bf16.
"""

import numpy as np
import ml_dtypes

NH, HD, C, H, W = 12, 64, 768, 64, 64
N = H * W            # 4096
G, QB = 4, 2         # head groups x query blocks = 8 cores
HPG = NH // G        # 3 heads per group
QL = N // QB         # 2048 queries per block
SCALE = HD ** -0.5

_prog_cache = {}


def _round_f32r(x):
    hi = x.astype(ml_dtypes.bfloat16).astype(np.float32)
    lo = (x - hi).astype(ml_dtypes.bfloat16).astype(np.float32)
    return np.ascontiguousarray(hi + lo)


def _pack6(w):
    # (768, M) -> [128, 6*M]: chunk c of the contraction at cols [c*M:(c+1)*M]
    m = w.shape[1]
    return np.ascontiguousarray(w.reshape(6, 128, m).transpose(1, 0, 2).reshape(128, 6 * m))


def _build_program(taps=False):
    import concourse.bacc as bacc
    import concourse.mybir as mybir
    import concourse.tile as tile
    from contextlib import ExitStack

    f32 = mybir.dt.float32
    f32r = mybir.dt.float32r
    bf16 = mybir.dt.bfloat16
    AF = mybir.ActivationFunctionType
    ADD = mybir.AluOpType.add

    nc = bacc.Bacc("TRN2", target_bir_lowering=False, debug=False)

    XT = nc.dram_tensor("xt", [C, N], f32r, kind="ExternalInput")
    XTQ = nc.dram_tensor("xtq", [C, QL], f32r, kind="ExternalInput")
    WA = nc.dram_tensor("wa", [128, 768], f32r, kind="ExternalInput")
    WC = nc.dram_tensor("wc", [128, 384], f32r, kind="ExternalInput")
    WVB = nc.dram_tensor("wvb", [128, 6 * 192], bf16, kind="ExternalInput")
    BVB = nc.dram_tensor("bvb", [128, 192], bf16, kind="ExternalInput")
    WQA = nc.dram_tensor("wqa", [128, 768], f32r, kind="ExternalInput")
    WQB = nc.dram_tensor("wqb", [128, 384], f32r, kind="ExternalInput")
    PW1 = nc.dram_tensor("pw1", [128, 768], f32r, kind="ExternalInput")
    PW2 = nc.dram_tensor("pw2", [64, 768], f32r, kind="ExternalInput")
    BA = nc.dram_tensor("ba", [128, 1], f32, kind="ExternalInput")
    BC_ = nc.dram_tensor("bc", [64, 1], f32, kind="ExternalInput")
    BQA = nc.dram_tensor("bqa", [128, 1], f32, kind="ExternalInput")
    BQB = nc.dram_tensor("bqb", [64, 1], f32, kind="ExternalInput")
    RHT = nc.dram_tensor("rht", [64, 32 * 64], bf16, kind="ExternalInput")
    RWT = nc.dram_tensor("rwt", [64, 64 * 64], bf16, kind="ExternalInput")
    IDKH = nc.dram_tensor("idkh", [64, N], f32r, kind="ExternalInput")
    OUT = nc.dram_tensor("out", [C, QL], f32, kind="ExternalOutput")

    VSTRIDE = HPG * 80  # 240 cols per token-tile in VN (80 per head: 64 V + 1 ones + 15 pad)
    if taps:
        TKAUG = nc.dram_tensor("t_kaug0", [128, N], f32r, kind="ExternalOutput")
        TQAUG = nc.dram_tensor("t_qaug0", [128, QL], f32r, kind="ExternalOutput")
        TEW2 = nc.dram_tensor("t_ew20", [128, QL], bf16, kind="ExternalOutput")
        TVN = nc.dram_tensor("t_vn", [128, 32 * VSTRIDE], bf16, kind="ExternalOutput")
        TE2 = nc.dram_tensor("t_e2", [128, 1024], bf16, kind="ExternalOutput")
        TOP = nc.dram_tensor("t_op", [65, 1024], f32, kind="ExternalOutput")
        TPRJ = nc.dram_tensor("t_prja", [128, QL], f32r, kind="ExternalOutput")

    with tile.TileContext(nc) as tc, ExitStack() as es:
        const = es.enter_context(tc.tile_pool(name="const", bufs=1))
        big = es.enter_context(tc.tile_pool(name="big", bufs=1))
        xp = es.enter_context(tc.tile_pool(name="xp", bufs=2))
        xbp = es.enter_context(tc.tile_pool(name="xbp", bufs=1))
        p1 = es.enter_context(tc.tile_pool(name="p1", bufs=2, space="PSUM"))
        scp = es.enter_context(tc.tile_pool(name="sc", bufs=3, space="PSUM"))
        ep = es.enter_context(tc.tile_pool(name="ep", bufs=2))
        nrm = es.enter_context(tc.tile_pool(name="nrm", bufs=1))
        ewfp = es.enter_context(tc.tile_pool(name="ewf", bufs=1))

        # ---- persistent tiles ----
        wA_t = const.tile([128, 768], f32r, tag="wA", name="wA")
        wC_t = const.tile([128, 384], f32r, tag="wC", name="wC")
        wvb_t = const.tile([128, 6 * 192], bf16, tag="wvb", name="wvb")
        bvb_t = const.tile([128, 192], bf16, tag="bvb", name="bvb")
        wqA_t = const.tile([128, 768], f32r, tag="wqA", name="wqA")
        wqB_t = const.tile([128, 384], f32r, tag="wqB", name="wqB")
        pw1_t = const.tile([128, 768], f32r, tag="pw1", name="pw1")
        pw2_t = const.tile([64, 768], f32r, tag="pw2", name="pw2")
        bA_t = const.tile([128, 1], f32, tag="bA", name="bA")
        bC_t = const.tile([64, 1], f32, tag="bC", name="bC")
        bqA_t = const.tile([128, 1], f32, tag="bqA", name="bqA")
        bqB_t = const.tile([64, 1], f32, tag="bqB", name="bqB")
        rhT_t = const.tile([64, 32 * 64], bf16, tag="rhT", name="rhT")
        rwT_t = const.tile([64, 64 * 64], bf16, tag="rwT", name="rwT")
        ones1 = const.tile([1, 64], f32, tag="ones1", name="ones1")

        for t_, d_ in [(wqA_t, WQA), (wqB_t, WQB), (bqA_t, BQA), (bqB_t, BQB),
                       (rhT_t, RHT), (rwT_t, RWT), (wA_t, WA), (wC_t, WC),
                       (wvb_t, WVB), (bvb_t, BVB), (bA_t, BA), (bC_t, BC_),
                       (pw1_t, PW1), (pw2_t, PW2)]:
            nc.sync.dma_start(t_[:], d_.ap())
        nc.vector.memset(ones1[:], 1.0)

        KAUG = [big.tile([128, N], f32r, tag=f"kaug{h}", name=f"kaug{h}") for h in range(HPG)]
        QAUG = [big.tile([128, QL], f32r, tag=f"qaug{h}", name=f"qaug{h}") for h in range(HPG)]
        
        EW2 = [big.tile([128, QL], bf16, tag=f"ew2{h}", name=f"ew2{h}") for h in range(HPG)]
        VN = big.tile([128, 32 * VSTRIDE], bf16, tag="vn", name="vn")
        PRJA = big.tile([128, QL], f32r, tag="prja", name="prja")
        PRJB = big.tile([64, QL], f32r, tag="prjb", name="prjb")

        for h in range(HPG):
            nc.sync.dma_start(KAUG[h][64:128, :], IDKH.ap())
        vn3 = VN[:].rearrange("p (t c) -> p t c", c=VSTRIDE)
        for h in range(HPG):
            nc.vector.memset(vn3[:, :, 64 + 80 * h], 1.0)

        # ---- Q projection over this core's block ----
        for t in range(4):
            xc = []
            for c in range(6):
                xt_ = xp.tile([128, 512], f32r, tag=f"x{c}", name=f"x{c}")
                nc.sync.dma_start(xt_[:], XTQ.ap()[128 * c:128 * c + 128, 512 * t:512 * t + 512])
                xc.append(xt_)
            sl = slice(512 * t, 512 * t + 512)
            ps = p1.tile([128, 512], f32, tag="p1", name="p1")
            for c in range(6):
                nc.tensor.matmul(ps[:], wqA_t[:, 128 * c:128 * c + 128], xc[c][:],
                                 start=(c == 0), stop=(c == 5))
            nc.vector.tensor_scalar(QAUG[0][0:64, sl], ps[0:64, :], bqA_t[0:64, :], None, ADD)
            nc.vector.tensor_scalar(QAUG[1][0:64, sl], ps[64:128, :], bqA_t[64:128, :], None, ADD)
            ps2 = p1.tile([64, 512], f32, tag="p1", name="p1b")
            for c in range(6):
                nc.tensor.matmul(ps2[:], wqB_t[:, 64 * c:64 * c + 64], xc[c][:],
                                 start=(c == 0), stop=(c == 5))
            nc.vector.tensor_scalar(QAUG[2][0:64, sl], ps2[:], bqB_t[:], None, ADD)
        # ---- RH^T into QAUG rows 64-127; RW^T -> exp -> EW2 (bf16 matmuls) ----
        for h in range(HPG):
            qb16 = ewfp.tile([64, QL], bf16, tag="qb16", name="qb16")
            nc.vector.tensor_copy(qb16[:], QAUG[h][0:64, :])
            for i4 in range(8):
                ps = p1.tile([64, 256], f32, tag="p1", name="p1rh")
                for k in range(4):
                    i = 4 * i4 + k
                    nc.tensor.matmul(ps[:, 64 * k:64 * k + 64],
                                     rhT_t[:, 64 * i:64 * i + 64],
                                     qb16[:, 64 * i:64 * i + 64],
                                     start=True, stop=True)
                nc.vector.tensor_copy(QAUG[h][64:128, 256 * i4:256 * i4 + 256], ps[:])
            ewf = ewfp.tile([64, QL], bf16, tag="ewf", name="ewf")
            qa = qb16[:].rearrange("p (i w) -> p w i", w=64)
            ef = ewf[:].rearrange("p (i w) -> p w i", w=64)
            for w4 in range(16):
                ps = p1.tile([64, 128], f32, tag="p1", name="p1rw")
                for k in range(4):
                    w = 4 * w4 + k
                    nc.tensor.matmul(ps[:, 32 * k:32 * k + 32],
                                     rwT_t[:, 64 * w:64 * w + 64], qa[:, w, :],
                                     start=True, stop=True)
                nc.vector.tensor_copy(
                    ef[:, 4 * w4:4 * w4 + 4, :],
                    ps[:].rearrange("p (k i) -> p k i", i=32))
            nc.scalar.activation(EW2[h][0:64, :], ewf[:], AF.Exp)
            nc.vector.tensor_copy(EW2[h][64:128, :], EW2[h][0:64, :])

        # ---- K projection + V natural layout, streaming all tokens ----
        for t in range(8):
            xc = []
            xb = []
            for c in range(6):
                xt_ = xp.tile([128, 512], f32r, tag=f"x{c}", name=f"x{c}")
                nc.sync.dma_start(xt_[:], XT.ap()[128 * c:128 * c + 128, 512 * t:512 * t + 512])
                xc.append(xt_)
                xb_ = xbp.tile([128, 512], bf16, tag=f"xb{c}", name=f"xb{c}")
                nc.gpsimd.tensor_copy(xb_[:], xt_[:])
                xb.append(xb_)
            sl = slice(512 * t, 512 * t + 512)
            ps = p1.tile([128, 512], f32, tag="p1", name="p1k")
            for c in range(6):
                nc.tensor.matmul(ps[:], wA_t[:, 128 * c:128 * c + 128], xc[c][:],
                                 start=(c == 0), stop=(c == 5))
            nc.vector.tensor_scalar(KAUG[0][0:64, sl], ps[0:64, :], bA_t[0:64, :], None, ADD)
            nc.vector.tensor_scalar(KAUG[1][0:64, sl], ps[64:128, :], bA_t[64:128, :], None, ADD)
            ps2 = p1.tile([64, 512], f32, tag="p1", name="p1k2")
            for c in range(6):
                nc.tensor.matmul(ps2[:], wC_t[:, 64 * c:64 * c + 64], xc[c][:],
                                 start=(c == 0), stop=(c == 5))
            nc.vector.tensor_scalar(KAUG[2][0:64, sl], ps2[:], bC_t[:], None, ADD)
            for s in range(4):
                tt = 4 * t + s
                pv = p1.tile([128, 192], f32, tag="p1", name="p1v")
                for c in range(6):
                    nc.tensor.matmul(pv[:], xb[c][:, 128 * s:128 * s + 128],
                                     wvb_t[:, 192 * c:192 * c + 192],
                                     start=(c == 0), stop=(c == 5))
                vdst = VN[:, VSTRIDE * tt:VSTRIDE * tt + VSTRIDE].rearrange(
                    "p (h c) -> p h c", c=80)[:, :, 0:64]
                nc.vector.tensor_tensor(
                    vdst, pv[:].rearrange("p (h c) -> p h c", c=64),
                    bvb_t[:].rearrange("p (h c) -> p h c", c=64), ADD)

        # ---- attention (chases the K/V stream via Tile deps) ----
        for h in range(HPG):
            for qc in range(2):
                q0 = 1024 * qc
                O_ps = scp.tile([65, 1024], f32, tag="sc", name="av")
                for kc in range(32):
                    S_ps = scp.tile([128, 1024], f32, tag="sc", name="sc")
                    for s in range(2):
                        nc.tensor.matmul(S_ps[:, 512 * s:512 * s + 512],
                                         KAUG[h][:, 128 * kc:128 * kc + 128],
                                         QAUG[h][:, q0 + 512 * s:q0 + 512 * s + 512],
                                         start=True, stop=True)
                    E1 = ep.tile([128, 1024], bf16, tag="e1", name="e1")
                    nc.scalar.activation(E1[:], S_ps[:], AF.Exp)
                    E2 = ep.tile([128, 1024], bf16, tag="e2", name="e2")
                    nc.vector.tensor_mul(E2[:], E1[:], EW2[h][:, q0:q0 + 1024])
                    if taps and h == 0 and qc == 0 and kc == 0:
                        nc.sync.dma_start(TE2.ap(), E2[:])
                    for s in range(2):
                        nc.tensor.matmul(O_ps[:, 512 * s:512 * s + 512],
                                         VN[:, VSTRIDE * kc + 80 * h:VSTRIDE * kc + 80 * h + 65],
                                         E2[:, 512 * s:512 * s + 512],
                                         start=(kc == 0), stop=(kc == 31))
                if taps and h == 0 and qc == 0:
                    topst = nrm.tile([65, 1024], f32, tag="topst", name="topst")
                    nc.vector.tensor_copy(topst[:], O_ps[:])
                    nc.sync.dma_start(TOP.ap(), topst[:])
                # copy O_ps to SBUF immediately so the next chunk can reuse the
                # PSUM slot; normalize from the SBUF copy
                O_sb = const.tile([65, 1024], f32, tag="rwT", name="osb")  # reuse rwT slot (dead after phase 1)
                nc.vector.tensor_copy(O_sb[:], O_ps[:])
                # normalize: 1/d via Ln+Exp (f32r out), broadcast via K=1 f32r matmul
                ln_t = nrm.tile([1, 1024], f32, tag="ln", name="ln")
                nc.scalar.activation(ln_t[:], O_sb[64:65, :], AF.Ln)
                rec_t = nrm.tile([1, 1024], f32r, tag="rec", name="rec")
                nc.scalar.activation(rec_t[:], ln_t[:], AF.Exp, scale=-1.0)
                onesr = nrm.tile([1, 64], f32r, tag="onesr", name="onesr")
                nc.vector.tensor_copy(onesr[:], ones1[:])
                B_sb = nrm.tile([64, 1024], f32, tag="bcs", name="bcs")
                for s in range(2):
                    B_ps = p1.tile([64, 512], f32, tag="p1", name="bcb")
                    nc.tensor.matmul(B_ps[:], onesr[:],
                                     rec_t[:, 512 * s:512 * s + 512], start=True, stop=True)
                    nc.vector.tensor_copy(B_sb[:, 512 * s:512 * s + 512], B_ps[:])
                dst = PRJA[64 * h:64 * h + 64, q0:q0 + 1024] if h < 2 else PRJB[0:64, q0:q0 + 1024]
                nc.vector.tensor_mul(dst, O_sb[0:64, :], B_sb[:])

        if taps:
            nc.sync.dma_start(TKAUG.ap(), KAUG[0][:])
            nc.sync.dma_start(TQAUG.ap(), QAUG[0][:])
            nc.sync.dma_start(TEW2.ap(), EW2[0][:])
            nc.sync.dma_start(TVN.ap(), VN[:])
            nc.sync.dma_start(TPRJ.ap(), PRJA[:])

        # ---- output projection (partial over this group's channels) ----
        if True:
            for m in range(6):
                for qc in range(2):
                    q0 = 1024 * qc
                    ps = scp.tile([128, 1024], f32, tag="sc", name="po")
                    for s in range(2):
                        nc.tensor.matmul(ps[:, 512 * s:512 * s + 512],
                                         pw1_t[:, 128 * m:128 * m + 128],
                                         PRJA[:, q0 + 512 * s:q0 + 512 * s + 512],
                                         start=True, stop=False)
                        nc.tensor.matmul(ps[:, 512 * s:512 * s + 512],
                                         pw2_t[:, 128 * m:128 * m + 128],
                                         PRJB[:, q0 + 512 * s:q0 + 512 * s + 512],
                                         start=False, stop=True)
                    ost = big.tile([128, 1024], f32, tag=f"ew2{(2 * m + qc) % 3}", name="ost")
                    nc.vector.tensor_copy(ost[:], ps[:])
                    nc.sync.dma_start(OUT.ap()[128 * m:128 * m + 128, q0:q0 + 1024], ost[:])

    nc.compile()
    return nc


def _host_inputs(x, qkv_w, qkv_b, proj_w, rel_pos_h, rel_pos_w):
    """Build the 8 per-core input maps."""
    xmat = np.ascontiguousarray(x.reshape(N, C))
    xT = _round_f32r(xmat.T.astype(np.float32))

    idx = np.arange(64)[:, None] - np.arange(64)[None, :] + 63
    rh_g = rel_pos_h[idx]            # (h, kh, c)
    rw_g = rel_pos_w[idx]            # (w, kw, c)
    rwT = np.ascontiguousarray(rw_g.transpose(2, 0, 1).reshape(64, 64 * 64)).astype(ml_dtypes.bfloat16)
    idkh = _round_f32r(
        (np.arange(64)[:, None] == (np.arange(N)[None, :] // 64)).astype(np.float32))

    in_maps = []
    for core in range(8):
        g, j = core // QB, core % QB
        cs = slice(192 * g, 192 * g + 192)
        wq = qkv_w[:, 0 * C:1 * C][:, cs]
        wk = qkv_w[:, 1 * C:2 * C][:, cs] * SCALE
        wv = qkv_w[:, 2 * C:3 * C][:, cs]
        bq = qkv_b[0 * C:1 * C][cs]
        bk = qkv_b[1 * C:2 * C][cs] * SCALE
        bv = qkv_b[2 * C:3 * C][cs]

        h0 = 32 * j
        rhT = np.ascontiguousarray(rh_g[h0:h0 + 32].transpose(2, 0, 1).reshape(64, 32 * 64)).astype(ml_dtypes.bfloat16)

        m = {
            "xt": xT,
            "xtq": np.ascontiguousarray(xT[:, QL * j:QL * j + QL]),
            "wa": _round_f32r(_pack6(wk[:, 0:128])),
            "wc": _round_f32r(_pack6(wk[:, 128:192])),
            "wvb": _pack6(wv).astype(ml_dtypes.bfloat16),
            "bvb": np.ascontiguousarray(
                np.broadcast_to(bv[None, :], (128, 192))).astype(ml_dtypes.bfloat16),
            "wqa": _round_f32r(_pack6(wq[:, 0:128])),
            "wqb": _round_f32r(_pack6(wq[:, 128:192])),
            "pw1": _round_f32r(proj_w[cs][0:128, :]),
            "pw2": _round_f32r(proj_w[cs][128:192, :]),
            "ba": np.ascontiguousarray(bk[0:128, None].astype(np.float32)),
            "bc": np.ascontiguousarray(bk[128:192, None].astype(np.float32)),
            "bqa": np.ascontiguousarray(bq[0:128, None].astype(np.float32)),
            "bqb": np.ascontiguousarray(bq[128:192, None].astype(np.float32)),
            "rht": rhT,
            "rwt": rwT,
            "idkh": idkh,
        }
        in_maps.append(m)
    return in_maps


def kernel(x, qkv_w, qkv_b, proj_w, proj_b, rel_pos_h, rel_pos_w):
    from concourse.bass_utils import run_bass_kernel_spmd

    x = np.asarray(x, dtype=np.float32)
    qkv_w = np.asarray(qkv_w, dtype=np.float32)
    qkv_b = np.asarray(qkv_b, dtype=np.float32)
    proj_w = np.asarray(proj_w, dtype=np.float32)
    proj_b = np.asarray(proj_b, dtype=np.float32)
    rel_pos_h = np.asarray(rel_pos_h, dtype=np.float32)
    rel_pos_w = np.asarray(rel_pos_w, dtype=np.float32)

    if "nc" not in _prog_cache:
        _prog_cache["nc"] = _build_program()
    nc = _prog_cache["nc"]

    in_maps = _host_inputs(x, qkv_w, qkv_b, proj_w, rel_pos_h, rel_pos_w)
    res = run_bass_kernel_spmd(nc, in_maps, core_ids=list(range(8)))

    out = np.zeros((N, C), dtype=np.float32)
    for core in range(8):
        g, j = core // QB, core % QB
        out[QL * j:QL * j + QL, :] += res.results[core]["out"].T
    out += proj_b[None, :]
    return out.reshape(1, H, W, C).astype(np.float32)
